# revision 1
# baseline (speedup 1.0000x reference)
"""CrossModalFusion Trainium2 kernel.

Reference computation (per batch b):
    q = rgb @ Wq + bq                 [S, H]
    k = pose @ Wk + bk                [S, H]
    v = pose @ Wv + bv                [S, H]
    attn = softmax(q @ k.T / sqrt(H)) [S, S]
    out  = attn @ v                   [S, H]
    proj = out @ Wp + bp              [S, D]
    x = rgb + gate * proj
    fused = LayerNorm(x) * gamma + beta

Sharding: pure data-parallel over batch B=32 across 8 NeuronCores
(4 batches per core), identical SPMD program, no collectives.

Per-core dataflow (per batch):
  - PE-transpose pose tiles -> poseT [d, S] (d on partitions).
  - kT[h, S] = Wk.T-chunks x poseT (feature-major), bias fused into the
    PSUM->SBUF copy on the scalar engine.
  - v[S, h] seq-major (lhsT for attn@v), bias via DVE, stored bf16.
  - per 512-column query block:
      rgbT/qT like above;
      scoresT[sk, sq] = kT-chunk.T @ qT  (transposed scores so the
        exp'd attention lands directly in the layout attn@v needs --
        no per-tile PE transposes of the attention matrix);
      exp on ACT with the 1/sqrt(H) scale fused, bf16 out, unnormalized;
      column sums via ones-row matmuls (reduction over partitions);
      outT[h, sq] = v-chunk x attnT (feature-major);
      proj[sq, d] = outT-chunk x Wp; softmax normalization and the gate
        are folded into the per-row scale applied at the residual;
      fused residual + LayerNorm (bn_stats/bn_aggr) and store.

All big matmuls run as float32r (fp32 data, reduced-precision PE mode,
full rate at free-dim >= 256). Attention weights and v are bf16.
"""

import numpy as np

B, S, D, H = 32, 2048, 400, 512
N_CORES = 8
B_LOC = B // N_CORES
LN_EPS = 1e-5
P = 128          # partitions
QBLK = 512       # query block (columns of scoresT)
NBLK = 512       # free-dim block for feature-major matmuls

WEIGHT_NAMES = ("Wq", "bq", "Wk", "bk", "Wv", "bv", "Wp", "bp",
                "ln_gamma", "ln_beta", "gate")


def _chunks(n, c=P):
    """[(start, size), ...] covering n in chunks of c."""
    return [(i, min(c, n - i)) for i in range(0, n, c)]


def build_nc(b_loc=B_LOC, s=S, d=D, h=H):
    import concourse.bass as bass
    import concourse.mybir as mybir
    import concourse.tile as tile
    from concourse import bacc
    from concourse.masks import make_identity

    def bcast(ap1d, p=P):
        """Broadcast a 1-D DRAM AP across p partitions (step-0 leading dim)."""
        return bass.AP(tensor=ap1d.tensor, offset=ap1d.offset,
                       ap=[[0, p]] + list(ap1d.ap))

    f32 = mybir.dt.float32
    f32r = mybir.dt.float32r
    bf16 = mybir.dt.bfloat16
    AF = mybir.ActivationFunctionType

    nt = s // P              # seq tiles
    nqb = s // QBLK          # query blocks
    tpb = QBLK // P          # seq tiles per query block
    nhb = h // NBLK          # feature-major free blocks for kT
    nht = h // P             # h tiles (partition chunks of H)
    dch = _chunks(d)         # d chunks (contraction for QKV)
    scale = 1.0 / float(np.sqrt(h))

    nc = bacc.Bacc("TRN2", target_bir_lowering=False, debug=False,
                   num_swdge_queues=4)

    rgb = nc.dram_tensor("rgb", [b_loc, s, d], f32, kind="ExternalInput").ap()
    pose = nc.dram_tensor("pose", [b_loc, s, d], f32, kind="ExternalInput").ap()
    Wq = nc.dram_tensor("Wq", [d, h], f32, kind="ExternalInput").ap()
    bq = nc.dram_tensor("bq", [h], f32, kind="ExternalInput").ap()
    Wk = nc.dram_tensor("Wk", [d, h], f32, kind="ExternalInput").ap()
    bk = nc.dram_tensor("bk", [h], f32, kind="ExternalInput").ap()
    Wv = nc.dram_tensor("Wv", [d, h], f32, kind="ExternalInput").ap()
    bv = nc.dram_tensor("bv", [h], f32, kind="ExternalInput").ap()
    Wp = nc.dram_tensor("Wp", [h, d], f32, kind="ExternalInput").ap()
    bp = nc.dram_tensor("bp", [d], f32, kind="ExternalInput").ap()
    gamma = nc.dram_tensor("ln_gamma", [d], f32, kind="ExternalInput").ap()
    beta = nc.dram_tensor("ln_beta", [d], f32, kind="ExternalInput").ap()
    gate = nc.dram_tensor("gate", [1], f32, kind="ExternalInput").ap()
    out = nc.dram_tensor("out", [b_loc, s, d], f32, kind="ExternalOutput").ap()

    from contextlib import ExitStack

    with tile.TileContext(nc) as tc:
        with ExitStack() as ctx:
            pool = lambda **kw: ctx.enter_context(tc.tile_pool(**kw))
            const = pool(name="const", bufs=1)
            wpool = pool(name="wpool", bufs=1)
            praw = pool(name="praw", bufs=9)
            ptp = pool(name="ptp", bufs=1)            # poseT
            ktp = pool(name="ktp", bufs=1)            # kT
            vtp = pool(name="vtp", bufs=1)            # v (bf16)
            rraw = pool(name="rraw", bufs=2 * tpb)
            rtp = pool(name="rtp", bufs=1)            # rgbT block
            qtp = pool(name="qtp", bufs=1)            # qT block
            atp = pool(name="atp", bufs=1)            # attnT (bf16)
            otp = pool(name="otp", bufs=1)            # outT block
            wstage = pool(name="wstage", bufs=2)
            small = pool(name="small", bufs=4)
            cspool = pool(name="cspool", bufs=1)
            ypool = pool(name="ypool", bufs=2)
            ps_sc = pool(name="ps_sc", bufs=2, space="PSUM")
            ps_mm = pool(name="ps_mm", bufs=3, space="PSUM")
            ps_tr = pool(name="ps_tr", bufs=2, space="PSUM")
            ps_cs = pool(name="ps_cs", bufs=1, space="PSUM")
            # ---- constants / weights (once per core) ----
            ident = const.tile([P, P], f32)
            make_identity(nc, ident)
            ones_sk = const.tile([P, 1], bf16)
            nc.vector.memset(ones_sk, 1.0)
            ones_11 = const.tile([1, 1], f32)
            nc.vector.memset(ones_11, 1.0)
            eps_sb = const.tile([P, 1], f32)
            nc.vector.memset(eps_sb, LN_EPS)

            warm = ps_tr.tile([P, P], f32, tag="tr")
            for _ in range(110):
                nc.tensor.matmul(warm, ident, ident, start=True, stop=True)

            copy_i = 0  # alternate PSUM->SBUF copies between DVE and ACT

            def psum_copy(dst, src):
                nonlocal copy_i
                copy_i += 1
                if copy_i % 3 == 0:
                    nc.scalar.copy(out=dst, in_=src)
                else:
                    nc.vector.tensor_copy(out=dst, in_=src)

            def transpose_in(dst_tp, raw, dst_col0):
                """PE-transpose raw [128, d] into dst_tp[:, c, dst_col0:+128]."""
                for c, (d0, dn) in enumerate(dch):
                    ps = ps_tr.tile([P, P], f32, tag="tr")
                    nc.tensor.transpose(ps[:dn, :], raw[:, d0:d0 + dn], ident)
                    psum_copy(dst_tp[:dn, c, dst_col0:dst_col0 + P], ps[:dn, :])

            def emit_pose_dma(b, t0, t1):
                tiles = []
                for t in range(t0, t1):
                    po = praw.tile([P, d], f32, tag="praw")
                    nc.sync.dma_start(out=po, in_=pose[b, t * P:(t + 1) * P, :])
                    tiles.append(po)
                return tiles

            def emit_pose_tr(poseT, tiles, t0):
                if poseT is None:
                    poseT = ptp.tile([P, len(dch), s], f32r, tag="poseT")
                for k, po in enumerate(tiles):
                    transpose_in(poseT, po, (t0 + k) * P)
                return poseT

            def emit_poseT(b, t0=0, t1=None, poseT=None):
                """pose[b] tiles [t0, t1) -> poseT (DMA + PE transposes)."""
                t1 = nt if t1 is None else t1
                return emit_pose_tr(poseT, emit_pose_dma(b, t0, t1), t0)

            def emit_qt(b, qb):
                """rgb block -> rgbT -> qT; returns (qT, rgb_raw tiles).

                Emitted between a block's scores and its attn@v so the PE
                has dense work while ACT runs the exp chain (keeps HAM at
                full clock)."""
                q0 = qb * QBLK
                rgbT = rtp.tile([P, len(dch), QBLK], f32r, tag="rgbT")
                rgb_raw = []
                for j in range(tpb):
                    rr = rraw.tile([P, d], f32, tag="rraw")
                    nc.sync.dma_start(
                        out=rr, in_=rgb[b, q0 + j * P:q0 + (j + 1) * P, :])
                    transpose_in(rgbT, rr, j * P)
                    rgb_raw.append(rr)
                qT = qtp.tile([P, nht, QBLK], f32r, tag="qT")
                for ht in range(nht):
                    ps = ps_mm.tile([P, QBLK], f32, tag="mm")
                    for c, (d0, dn) in enumerate(dch):
                        nc.tensor.matmul(
                            ps,
                            wq_sb[:dn, c, ht * P:(ht + 1) * P],
                            rgbT[:dn, c, :],
                            start=(c == 0), stop=(c == len(dch) - 1),
                        )
                    nc.scalar.activation(
                        out=qT[:, ht, :], in_=ps,
                        func=AF.Identity, bias=bq_sb[:, ht:ht + 1],
                    )
                # residual base: rgb += gate*bp (after transposes read rgb)
                for j in range(tpb):
                    nc.vector.tensor_add(
                        out=rgb_raw[j], in0=rgb_raw[j], in1=bpg_bc)
                return qT, rgb_raw

            poseT = emit_poseT(0)  # pose DMAs launch before the weight loads

            # fp32r matmul operands must be written "rounded": stage the
            # DMA'd fp32 weights and round them into f32r tiles via copies.
            wq_sb = wpool.tile([P, len(dch), h], f32r)
            wk_sb = wpool.tile([P, len(dch), h], f32r)
            wv_sb = wpool.tile([P, len(dch), h], f32r)
            for dst, W in ((wq_sb, Wq), (wk_sb, Wk), (wv_sb, Wv)):
                wst = wstage.tile([P, len(dch), h], f32, tag="wst")
                for c, (d0, dn) in enumerate(dch):
                    nc.gpsimd.dma_start(out=wst[:dn, c, :], in_=W[d0:d0 + dn, :])
                    nc.vector.tensor_copy(out=dst[:dn, c, :], in_=wst[:dn, c, :])
            wp_sb = wpool.tile([P, nht, d], f32r)
            wst = wstage.tile([P, len(dch), h], f32, tag="wst")
            for t in range(nht):
                nc.gpsimd.dma_start(out=wst[:, t, :d], in_=Wp[t * P:(t + 1) * P, :])
            nc.vector.tensor_copy(out=wp_sb, in_=wst[:, :, :d])

            # per-partition bias chunks: bq_sb[p, t] = bq[t*128 + p]
            bq_sb = wpool.tile([P, nht], f32)
            nc.gpsimd.dma_start(out=bq_sb, in_=bq.rearrange("(t p) -> p t", p=P))
            bk_sb = wpool.tile([P, nht], f32)
            nc.gpsimd.dma_start(out=bk_sb, in_=bk.rearrange("(t p) -> p t", p=P))
            # free-dim broadcasts
            bv_bc = wpool.tile([P, h], f32)
            nc.gpsimd.dma_start(out=bv_bc, in_=bcast(bv))
            bp_bc = wpool.tile([P, d], f32)
            nc.gpsimd.dma_start(out=bp_bc, in_=bcast(bp))
            gamma_bc = wpool.tile([P, d], f32)
            nc.gpsimd.dma_start(out=gamma_bc, in_=bcast(gamma))
            beta_bc = wpool.tile([P, d], f32)
            nc.gpsimd.dma_start(out=beta_bc, in_=bcast(beta))
            gate_sb = wpool.tile([P, 1], f32)
            nc.gpsimd.dma_start(out=gate_sb, in_=bcast(gate))
            # bpg = gate * bp (added to rgb once per row tile)
            bpg_bc = wpool.tile([P, d], f32)
            nc.vector.tensor_scalar_mul(out=bpg_bc, in0=bp_bc, scalar1=gate_sb)

            for b in range(b_loc):
                # ================= phase A: kT, v =================
                kT = ktp.tile([P, nht, s], f32r, tag="kT")
                for ht in range(nht):
                    for nb in range(s // NBLK):
                        ps = ps_mm.tile([P, NBLK], f32, tag="mm")
                        for c, (d0, dn) in enumerate(dch):
                            nc.tensor.matmul(
                                ps,
                                wk_sb[:dn, c, ht * P:(ht + 1) * P],
                                poseT[:dn, c, nb * NBLK:(nb + 1) * NBLK],
                                start=(c == 0), stop=(c == len(dch) - 1),
                            )
                        # bias add fused into the PSUM->SBUF copy (per-partition)
                        nc.scalar.activation(
                            out=kT[:, ht, nb * NBLK:(nb + 1) * NBLK], in_=ps,
                            func=AF.Identity, bias=bk_sb[:, ht:ht + 1],
                        )

                v_sb = vtp.tile([P, nt, h], bf16, tag="v")
                for t in range(nt):
                    ps = ps_mm.tile([P, h], f32, tag="mm")
                    for c, (d0, dn) in enumerate(dch):
                        nc.tensor.matmul(
                            ps,
                            poseT[:dn, c, t * P:(t + 1) * P],
                            wv_sb[:dn, c, :],
                            start=(c == 0), stop=(c == len(dch) - 1),
                        )
                    # v = psum + bv (free-dim bias), cast to bf16
                    nc.vector.scalar_tensor_tensor(
                        out=v_sb[:, t, :], in0=ps, scalar=1.0, in1=bv_bc,
                        op0=mybir.AluOpType.mult, op1=mybir.AluOpType.add,
                    )

                # ============ phase B: query blocks (pipelined) ============
                qstate = emit_qt(b, 0)
                for qb in range(nqb):
                    q0 = qb * QBLK
                    qT, rgb_raw = qstate

                    # scoresT tiles [sk 128, sq QBLK]; exp -> attnT (bf16)
                    attnT = atp.tile([P, nt, QBLK], bf16, tag="attnT")
                    for c in range(nt):
                        ps = ps_sc.tile([P, QBLK], f32, tag="sc")
                        for ht in range(nht):
                            nc.tensor.matmul(
                                ps,
                                kT[:, ht, c * P:(c + 1) * P],
                                qT[:, ht, :],
                                start=(ht == 0), stop=(ht == nht - 1),
                            )
                        nc.scalar.activation(
                            out=attnT[:, c, :], in_=ps, func=AF.Exp, scale=scale)

                    # pipeline filler: PE builds the next qT (or the next
                    # batch's poseT) while ACT runs this block's exp chain.
                    # The next batch's pose prefetch is split across the last
                    # two blocks, and each half's DMAs are issued ahead of
                    # the PE work that fills the same window, so the
                    # transposes never stall on the loads.
                    if qb + 1 < nqb:
                        prefetch = (qb == nqb - 2 and b + 1 < b_loc)
                        if prefetch:
                            ptiles = emit_pose_dma(b + 1, 0, nt // 2)
                        qstate = emit_qt(b, qb + 1)
                        if prefetch:
                            next_poseT = emit_pose_tr(None, ptiles, 0)
                            ptiles2 = emit_pose_dma(b + 1, nt // 2, nt)
                    elif b + 1 < b_loc:
                        if nqb >= 2:
                            next_poseT = emit_pose_tr(
                                next_poseT, ptiles2, nt // 2)
                        else:
                            next_poseT = emit_poseT(b + 1)

                    # outT[h, sq] = sum_c v[c].T-chunk @ attnT[c]
                    outT = otp.tile([P, nht, QBLK], f32r, tag="outT")
                    for ht in range(nht):
                        ps = ps_mm.tile([P, QBLK], f32, tag="mm")
                        for c in range(nt):
                            nc.tensor.matmul(
                                ps,
                                v_sb[:, c, ht * P:(ht + 1) * P],
                                attnT[:, c, :],
                                start=(c == 0), stop=(c == nt - 1),
                            )
                        psum_copy(outT[:, ht, :], ps)

                    # column sums of exp(scoresT): ones.T @ attnT, [1, QBLK]
                    cs = ps_cs.tile([1, QBLK], f32, tag="cs")
                    for c in range(nt):
                        nc.tensor.matmul(
                            cs, ones_sk, attnT[:, c, :],
                            start=(c == 0), stop=(c == nt - 1),
                        )
                    csum = cspool.tile([1, QBLK], f32, tag="csum")
                    nc.vector.tensor_copy(out=csum, in_=cs)

                    # per-row scale: gr = gate / colsum, materialized per tile
                    for j in range(tpb):
                        # rank-1 matmul scatters csum chunk across partitions
                        pst = ps_tr.tile([P, 1], f32, tag="tr")
                        nc.tensor.matmul(
                            pst, csum[0:1, j * P:(j + 1) * P], ones_11,
                            start=True, stop=True,
                        )
                        rec = small.tile([P, 1], f32, tag="rec")
                        nc.vector.reciprocal(out=rec, in_=pst)
                        gr = small.tile([P, 1], f32, tag="gr")
                        nc.vector.tensor_mul(out=gr, in0=rec, in1=gate_sb)

                        psp = ps_mm.tile([P, d], f32, tag="mm")
                        for ht in range(nht):
                            nc.tensor.matmul(
                                psp,
                                outT[:, ht, j * P:(j + 1) * P],
                                wp_sb[:, ht, :],
                                start=(ht == 0), stop=(ht == nht - 1),
                            )
                        # x = gr * proj + (rgb + gate*bp)
                        x = ypool.tile([P, d], f32, tag="x")
                        nc.vector.scalar_tensor_tensor(
                            out=x, in0=psp, scalar=gr, in1=rgb_raw[j],
                            op0=mybir.AluOpType.mult, op1=mybir.AluOpType.add,
                        )
                        # LayerNorm
                        stats = small.tile([P, 6], f32, tag="stats")
                        nc.vector.bn_stats(out=stats, in_=x)
                        mv = small.tile([P, 2], f32, tag="mv")
                        nc.vector.bn_aggr(out=mv, in_=stats)
                        sd = small.tile([P, 1], f32, tag="sd")
                        nc.scalar.activation(
                            out=sd, in_=mv[:, 1:2], func=AF.Sqrt, bias=eps_sb)
                        rstd = small.tile([P, 1], f32, tag="rstd")
                        nc.vector.reciprocal(out=rstd, in_=sd)
                        nc.vector.tensor_scalar(
                            out=x, in0=x, scalar1=mv[:, 0:1], scalar2=rstd,
                            op0=mybir.AluOpType.subtract, op1=mybir.AluOpType.mult,
                        )
                        nc.vector.tensor_mul(out=x, in0=x, in1=gamma_bc)
                        nc.vector.tensor_add(out=x, in0=x, in1=beta_bc)
                        nc.sync.dma_start(
                            out=out[b, q0 + j * P:q0 + (j + 1) * P, :], in_=x)

                if b + 1 < b_loc:
                    poseT = next_poseT

    nc.compile()
    return nc


_CACHE = {}


def kernel(**inputs):
    from concourse.bass_utils import run_bass_kernel_spmd

    if "nc" not in _CACHE:
        _CACHE["nc"] = build_nc()
    nc = _CACHE["nc"]

    weights = {k: np.ascontiguousarray(inputs[k], dtype=np.float32)
               for k in WEIGHT_NAMES}
    rgb = np.ascontiguousarray(inputs["rgb"], dtype=np.float32)
    pose = np.ascontiguousarray(inputs["pose"], dtype=np.float32)

    in_maps = []
    for i in range(N_CORES):
        m = dict(weights)
        m["rgb"] = np.ascontiguousarray(rgb[i * B_LOC:(i + 1) * B_LOC])
        m["pose"] = np.ascontiguousarray(pose[i * B_LOC:(i + 1) * B_LOC])
        in_maps.append(m)

    res = run_bass_kernel_spmd(nc, in_maps, list(range(N_CORES))).results
    return np.concatenate([res[i]["out"] for i in range(N_CORES)], axis=0)



# revision 18
# speedup vs baseline: 1.8658x; 1.8658x over previous
"""CrossModalFusion Trainium2 kernel (fp8 DoubleRow edition).

Reference computation (per batch b):
    q = rgb @ Wq + bq                 [S, H]
    k = pose @ Wk + bk                [S, H]
    v = pose @ Wv + bv                [S, H]
    attn = softmax(q @ k.T / sqrt(H)) [S, S]
    out  = attn @ v                   [S, H]
    proj = out @ Wp + bp              [S, D]
    x = rgb + gate * proj
    fused = LayerNorm(x) * gamma + beta

Sharding: pure data-parallel over batch B=32 across 8 NeuronCores
(4 batches per core), identical SPMD program, no collectives.

All large matmuls run in fp8e4 (e4m3) with MatmulPerfMode.DoubleRow,
which streams two K-tiles per pass (out = A.T@X0 + B.T@X1) at 2x the
bf16/f32r row rate.  Range management (e4m3 overflows to inf above
240):
  - Wq/Wk/Wv/Wp are prescaled by 16 into fp8 normal range; the factor
    is removed again via scale=1/16 in the PSUM->SBUF copies (q/k/v)
    and via the per-row normalization scale (proj).
  - attnT = exp(scores/sqrt(H) - ln 8): the 1/8 keeps exp well under
    240 even for 6-sigma scores; it cancels exactly in the softmax
    normalization (colsum is computed from the same attnT).
  - q/k/v/pose/rgb values are ~N(0,1), outT ~ +-100 -- all in range.

Per-core dataflow (per batch):
  - cast pose tiles to fp8, PE-transpose -> poseT [d, S] (d on
    partitions, 4 chunks of K=100 so d=400 needs no zero padding).
  - kT[h, S] = Wk-chunks x poseT (feature-major, 2 DoubleRow duals),
    bias+unscale fused into the PSUM->SBUF copy on the scalar engine.
  - v[S, h] seq-major (lhsT for attn@v), bias via DVE, fp8 out.
  - per 512-column query block:
      rgbT/qT like above;
      scoresT[sk, sq] = kT-duals.T @ qT (2 duals over H);
      exp on ACT with scale and -ln8 bias fused, fp8 out, unnormalized;
      column sums via ones-duals (reduction over partitions);
      outT[h, sq] = v-duals x attnT (8 duals over S);
      proj[sq, d] = outT-duals x Wp (2 duals over H); softmax
        normalization, the gate and the 1/16 weight prescale are folded
        into the per-row scale applied at the residual;
      fused residual + LayerNorm (bn_stats/bn_aggr) and store.
"""

import numpy as np

B, S, D, H = 32, 2048, 400, 512
N_CORES = 8
B_LOC = B // N_CORES
LN_EPS = 1e-5
P = 128          # partitions
QBLK = 512       # query block (columns of scoresT)
NBLK = 512       # free-dim block for feature-major matmuls
DK = 100         # d-chunk partition size (4*100 = 400, no padding)
WSCALE = 16.0    # fp8 prescale on weights
VSCALE = 4.0     # extra downscale on v (keeps outT under fp8 overflow)
EXPB = -np.log(8.0)  # exp output scale (cancels in normalization)

WEIGHT_NAMES = ("Wq", "bq", "Wk", "bk", "Wv", "bv", "Wp", "bp",
                "ln_gamma", "ln_beta", "gate")


def build_nc(b_loc=B_LOC, s=S, d=D, h=H):
    import concourse.bass as bass
    import concourse.mybir as mybir
    import concourse.tile as tile
    from concourse import bacc
    from concourse.masks import make_identity

    def bcast(ap1d, p=P):
        """Broadcast a 1-D DRAM AP across p partitions (step-0 leading dim)."""
        return bass.AP(tensor=ap1d.tensor, offset=ap1d.offset,
                       ap=[[0, p]] + list(ap1d.ap))

    f32 = mybir.dt.float32
    bf16 = mybir.dt.bfloat16
    fp8 = mybir.dt.float8e4
    AF = mybir.ActivationFunctionType
    DR = mybir.MatmulPerfMode.DoubleRow

    nt = s // P              # seq tiles
    nqb = s // QBLK          # query blocks
    tpb = QBLK // P          # seq tiles per query block
    nht = h // P             # h tiles (partition chunks of H)
    nhd = nht // 2           # h duals
    ndc = d // DK            # d chunks (K=100)
    ndd = ndc // 2           # d duals
    ncd = nt // 2            # seq duals (attn@v contraction)
    scale = 1.0 / float(np.sqrt(h))

    nc = bacc.Bacc("TRN2", target_bir_lowering=False, debug=False,
                   num_swdge_queues=4)

    rgb = nc.dram_tensor("rgb", [b_loc, s, d], f32, kind="ExternalInput").ap()
    pose = nc.dram_tensor("pose", [b_loc, s, d], f32, kind="ExternalInput").ap()
    Wq = nc.dram_tensor("Wq", [d, h], f32, kind="ExternalInput").ap()
    bq = nc.dram_tensor("bq", [h], f32, kind="ExternalInput").ap()
    Wk = nc.dram_tensor("Wk", [d, h], f32, kind="ExternalInput").ap()
    bk = nc.dram_tensor("bk", [h], f32, kind="ExternalInput").ap()
    Wv = nc.dram_tensor("Wv", [d, h], f32, kind="ExternalInput").ap()
    bv = nc.dram_tensor("bv", [h], f32, kind="ExternalInput").ap()
    Wp = nc.dram_tensor("Wp", [h, d], f32, kind="ExternalInput").ap()
    bp = nc.dram_tensor("bp", [d], f32, kind="ExternalInput").ap()
    gamma = nc.dram_tensor("ln_gamma", [d], f32, kind="ExternalInput").ap()
    beta = nc.dram_tensor("ln_beta", [d], f32, kind="ExternalInput").ap()
    gate = nc.dram_tensor("gate", [1], f32, kind="ExternalInput").ap()
    out = nc.dram_tensor("out", [b_loc, s, d], f32, kind="ExternalOutput").ap()

    from contextlib import ExitStack

    with tile.TileContext(nc) as tc:
        with ExitStack() as ctx:
            pool = lambda **kw: ctx.enter_context(tc.tile_pool(**kw))
            const = pool(name="const", bufs=1)
            wpool = pool(name="wpool", bufs=1)
            praw = pool(name="praw", bufs=9)
            p8pool = pool(name="p8pool", bufs=3)      # fp8 casts of raw tiles
            ptp = pool(name="ptp", bufs=1)            # poseT (fp8)
            ktp = pool(name="ktp", bufs=1)            # kT (fp8)
            vtp = pool(name="vtp", bufs=1)            # v (fp8)
            rraw = pool(name="rraw", bufs=2 * tpb)
            rtp = pool(name="rtp", bufs=1)            # rgbT block (fp8)
            qtp = pool(name="qtp", bufs=1)            # qT block (fp8)
            atp = pool(name="atp", bufs=1)            # attnT (fp8)
            otp = pool(name="otp", bufs=1)            # outT block (fp8)
            wstage = pool(name="wstage", bufs=2)
            small = pool(name="small", bufs=4)
            cspool = pool(name="cspool", bufs=1)
            ypool = pool(name="ypool", bufs=2)
            ps_sc = pool(name="ps_sc", bufs=2, space="PSUM")
            ps_mm = pool(name="ps_mm", bufs=3, space="PSUM")
            ps_tr = pool(name="ps_tr", bufs=2, space="PSUM")
            ps_cs = pool(name="ps_cs", bufs=1, space="PSUM")
            # ---- constants / weights (once per core) ----
            ident16 = const.tile([P, P], bf16)
            make_identity(nc, ident16)
            ones8 = const.tile([P, 2, 32], fp8)
            nc.vector.memset(ones8, 1.0)
            ones_11 = const.tile([1, 1], f32)
            nc.vector.memset(ones_11, 1.0)
            eps_sb = const.tile([P, 1], f32)
            nc.vector.memset(eps_sb, LN_EPS)
            expb_sb = const.tile([P, 1], f32)
            nc.vector.memset(expb_sb, EXPB)

            warm = ps_tr.tile([P, P], f32, tag="tr")
            for _ in range(110):
                nc.tensor.matmul(warm, ident16, ident16, start=True, stop=True)

            copy_i = 0  # alternate PSUM->SBUF copies between DVE and ACT

            def psum_copy(dst, src):
                nonlocal copy_i
                copy_i += 1
                if copy_i % 3 == 0:
                    nc.scalar.copy(out=dst, in_=src)
                else:
                    nc.vector.tensor_copy(out=dst, in_=src)

            def transpose_in(dst_tp, raw, dst_col0):
                """Cast raw [128, d] to bf16, PE-transpose, cast to fp8 in
                the PSUM->SBUF copy into dst_tp[:DK, c, dst_col0:+128]."""
                r16 = p8pool.tile([P, d], bf16, tag="r16")
                nc.vector.tensor_copy(out=r16, in_=raw)
                for c in range(ndc):
                    ps = ps_tr.tile([P, P], bf16, tag="tr")
                    nc.tensor.transpose(
                        ps[:DK, :], r16[:, c * DK:(c + 1) * DK], ident16)
                    psum_copy(dst_tp[:DK, c, dst_col0:dst_col0 + P],
                              ps[:DK, :])

            def emit_pose_dma(b, t0, t1):
                tiles = []
                for t in range(t0, t1):
                    po = praw.tile([P, d], f32, tag="praw")
                    nc.sync.dma_start(out=po, in_=pose[b, t * P:(t + 1) * P, :])
                    tiles.append(po)
                return tiles

            def emit_pose_tr(poseT, tiles, t0):
                if poseT is None:
                    poseT = ptp.tile([P, ndc, s], fp8, tag="poseT")
                for k, po in enumerate(tiles):
                    transpose_in(poseT, po, (t0 + k) * P)
                return poseT

            def emit_poseT(b, t0=0, t1=None, poseT=None):
                """pose[b] tiles [t0, t1) -> poseT (DMA + PE transposes)."""
                t1 = nt if t1 is None else t1
                return emit_pose_tr(poseT, emit_pose_dma(b, t0, t1), t0)

            def emit_qt(b, qb):
                """rgb block -> rgbT -> qT; returns (qT, rgb_raw tiles).

                Emitted between a block's scores and its attn@v so the PE
                has dense work while ACT runs the exp chain."""
                q0 = qb * QBLK
                rgbT = rtp.tile([P, ndc, QBLK], fp8, tag="rgbT")
                rgb_raw = []
                for j in range(tpb):
                    rr = rraw.tile([P, d], f32, tag="rraw")
                    nc.sync.dma_start(
                        out=rr, in_=rgb[b, q0 + j * P:q0 + (j + 1) * P, :])
                    transpose_in(rgbT, rr, j * P)
                    rgb_raw.append(rr)
                qT = qtp.tile([P, nht, QBLK], fp8, tag="qT")
                for ht in range(nht):
                    ps = ps_mm.tile([P, QBLK], f32, tag="mm")
                    for dd in range(ndd):
                        nc.tensor.matmul(
                            ps,
                            wq8[:DK, 2 * dd:2 * dd + 2, ht * P:(ht + 1) * P],
                            rgbT[:DK, 2 * dd:2 * dd + 2, :],
                            start=(dd == 0), stop=(dd == ndd - 1),
                            perf_mode=DR,
                        )
                    # q = psum/WSCALE + bq, cast to fp8
                    nc.scalar.activation(
                        out=qT[:, ht, :], in_=ps,
                        func=AF.Identity, scale=1.0 / WSCALE,
                        bias=bq_sb[:, ht:ht + 1],
                    )
                # residual base: rgb += gate*bp (after transposes read rgb)
                for j in range(tpb):
                    nc.vector.tensor_add(
                        out=rgb_raw[j], in0=rgb_raw[j], in1=bpg_bc)
                return qT, rgb_raw

            poseT = emit_poseT(0)  # pose DMAs launch before the weight loads

            # weights: DMA fp32 chunks, cast to fp8 with WSCALE prescale.
            wq8 = wpool.tile([P, ndc, h], fp8)
            wk8 = wpool.tile([P, ndc, h], fp8)
            wv8 = wpool.tile([P, ndc, h], fp8)
            for dst, W in ((wq8, Wq), (wk8, Wk), (wv8, Wv)):
                wst = wstage.tile([P, ndc, h], f32, tag="wst")
                for c in range(ndc):
                    nc.gpsimd.dma_start(
                        out=wst[:DK, c, :], in_=W[c * DK:(c + 1) * DK, :])
                    nc.vector.tensor_scalar(
                        out=dst[:DK, c, :], in0=wst[:DK, c, :],
                        scalar1=WSCALE, scalar2=None,
                        op0=mybir.AluOpType.mult,
                    )
            wp8 = wpool.tile([P, nht, d], fp8)
            wst = wstage.tile([P, nht, d], f32, tag="wstp")
            for t in range(nht):
                nc.gpsimd.dma_start(
                    out=wst[:, t, :], in_=Wp[t * P:(t + 1) * P, :])
            nc.vector.tensor_scalar(
                out=wp8, in0=wst, scalar1=WSCALE, scalar2=None,
                op0=mybir.AluOpType.mult,
            )

            # per-partition bias chunks: bq_sb[p, t] = bq[t*128 + p]
            bq_sb = wpool.tile([P, nht], f32)
            nc.gpsimd.dma_start(out=bq_sb, in_=bq.rearrange("(t p) -> p t", p=P))
            bk_sb = wpool.tile([P, nht], f32)
            nc.gpsimd.dma_start(out=bk_sb, in_=bk.rearrange("(t p) -> p t", p=P))
            # free-dim broadcasts
            bv_bc = wpool.tile([P, h], f32)
            nc.gpsimd.dma_start(out=bv_bc, in_=bcast(bv))
            bv4_bc = wpool.tile([P, h], f32)
            nc.vector.tensor_scalar(
                out=bv4_bc, in0=bv_bc, scalar1=1.0 / VSCALE, scalar2=None,
                op0=mybir.AluOpType.mult,
            )
            bp_bc = wpool.tile([P, d], f32)
            nc.gpsimd.dma_start(out=bp_bc, in_=bcast(bp))
            gamma_bc = wpool.tile([P, d], f32)
            nc.gpsimd.dma_start(out=gamma_bc, in_=bcast(gamma))
            beta_bc = wpool.tile([P, d], f32)
            nc.gpsimd.dma_start(out=beta_bc, in_=bcast(beta))
            gate_sb = wpool.tile([P, 1], f32)
            nc.gpsimd.dma_start(out=gate_sb, in_=bcast(gate))
            # bpg = gate * bp (added to rgb once per row tile)
            bpg_bc = wpool.tile([P, d], f32)
            nc.vector.tensor_scalar_mul(out=bpg_bc, in0=bp_bc, scalar1=gate_sb)
            # gr carries gate * VSCALE/WSCALE (leftover fp8 prescales)
            gate16 = wpool.tile([P, 1], f32)
            nc.vector.tensor_scalar(
                out=gate16, in0=gate_sb, scalar1=VSCALE / WSCALE, scalar2=None,
                op0=mybir.AluOpType.mult,
            )

            for b in range(b_loc):
                # ================= phase A: kT, v =================
                kT = ktp.tile([P, nht, s], fp8, tag="kT")
                for ht in range(nht):
                    for nb in range(s // NBLK):
                        ps = ps_mm.tile([P, NBLK], f32, tag="mm")
                        for dd in range(ndd):
                            nc.tensor.matmul(
                                ps,
                                wk8[:DK, 2 * dd:2 * dd + 2,
                                    ht * P:(ht + 1) * P],
                                poseT[:DK, 2 * dd:2 * dd + 2,
                                      nb * NBLK:(nb + 1) * NBLK],
                                start=(dd == 0), stop=(dd == ndd - 1),
                                perf_mode=DR,
                            )
                        # k = psum/WSCALE + bk fused into the PSUM->SBUF copy
                        nc.scalar.activation(
                            out=kT[:, ht, nb * NBLK:(nb + 1) * NBLK], in_=ps,
                            func=AF.Identity, scale=1.0 / WSCALE,
                            bias=bk_sb[:, ht:ht + 1],
                        )

                v_sb = vtp.tile([P, nt, h], fp8, tag="v")
                for t in range(nt):
                    ps = ps_mm.tile([P, h], f32, tag="mm")
                    for dd in range(ndd):
                        nc.tensor.matmul(
                            ps,
                            poseT[:DK, 2 * dd:2 * dd + 2, t * P:(t + 1) * P],
                            wv8[:DK, 2 * dd:2 * dd + 2, :],
                            start=(dd == 0), stop=(dd == ndd - 1),
                            perf_mode=DR,
                        )
                    # v = (psum/WSCALE + bv)/VSCALE (free-dim bias), fp8 out.
                    # The extra 1/VSCALE keeps outT = attnT@v under fp8's
                    # 240 overflow (rows with a dominant weight reach ~450
                    # otherwise); it is undone in gr.
                    nc.vector.scalar_tensor_tensor(
                        out=v_sb[:, t, :], in0=ps,
                        scalar=1.0 / (WSCALE * VSCALE), in1=bv4_bc,
                        op0=mybir.AluOpType.mult, op1=mybir.AluOpType.add,
                    )

                # ============ phase B: query blocks (pipelined) ============
                qstate = emit_qt(b, 0)
                for qb in range(nqb):
                    q0 = qb * QBLK
                    qT, rgb_raw = qstate

                    # scoresT tiles [sk 128, sq QBLK]; exp -> attnT (fp8)
                    attnT = atp.tile([P, nt, QBLK], fp8, tag="attnT")
                    for c in range(nt):
                        ps = ps_sc.tile([P, QBLK], f32, tag="sc")
                        for hd in range(nhd):
                            nc.tensor.matmul(
                                ps,
                                kT[:, 2 * hd:2 * hd + 2, c * P:(c + 1) * P],
                                qT[:, 2 * hd:2 * hd + 2, :],
                                start=(hd == 0), stop=(hd == nhd - 1),
                                perf_mode=DR,
                            )
                        nc.scalar.activation(
                            out=attnT[:, c, :], in_=ps, func=AF.Exp,
                            scale=scale, bias=expb_sb)

                    # pipeline filler: PE builds the next qT (or the next
                    # batch's poseT) while ACT runs this block's exp chain.
                    if qb + 1 < nqb:
                        prefetch = (qb == nqb - 2 and b + 1 < b_loc)
                        if prefetch:
                            ptiles = emit_pose_dma(b + 1, 0, nt // 2)
                        qstate = emit_qt(b, qb + 1)
                        if prefetch:
                            next_poseT = emit_pose_tr(None, ptiles, 0)
                            ptiles2 = emit_pose_dma(b + 1, nt // 2, nt)
                    elif b + 1 < b_loc:
                        if nqb >= 2:
                            next_poseT = emit_pose_tr(
                                next_poseT, ptiles2, nt // 2)
                        else:
                            next_poseT = emit_poseT(b + 1)

                    # outT[h, sq] = sum_cd v-dual @ attnT-dual
                    outT = otp.tile([P, nht, QBLK], fp8, tag="outT")
                    for ht in range(nht):
                        ps = ps_mm.tile([P, QBLK], f32, tag="mm")
                        for cd in range(ncd):
                            nc.tensor.matmul(
                                ps,
                                v_sb[:, 2 * cd:2 * cd + 2, ht * P:(ht + 1) * P],
                                attnT[:, 2 * cd:2 * cd + 2, :],
                                start=(cd == 0), stop=(cd == ncd - 1),
                                perf_mode=DR,
                            )
                        psum_copy(outT[:, ht, :], ps)

                    # column sums of attnT: ones-duals, [1, QBLK]
                    cs = ps_cs.tile([1, QBLK], f32, tag="cs")
                    for cd in range(ncd):
                        nc.tensor.matmul(
                            cs, ones8[:, :, 0:1], attnT[:, 2 * cd:2 * cd + 2, :],
                            start=(cd == 0), stop=(cd == ncd - 1),
                            perf_mode=DR,
                        )
                    csum = cspool.tile([1, QBLK], f32, tag="csum")
                    nc.vector.tensor_copy(out=csum, in_=cs)

                    # per-row scale: gr = gate/(WSCALE*colsum), per tile
                    for j in range(tpb):
                        # rank-1 matmul scatters csum chunk across partitions
                        pst = ps_tr.tile([P, 1], f32, tag="tr")
                        nc.tensor.matmul(
                            pst, csum[0:1, j * P:(j + 1) * P], ones_11,
                            start=True, stop=True,
                        )
                        rec = small.tile([P, 1], f32, tag="rec")
                        nc.vector.reciprocal(out=rec, in_=pst)
                        gr = small.tile([P, 1], f32, tag="gr")
                        nc.vector.tensor_mul(out=gr, in0=rec, in1=gate16)

                        psp = ps_mm.tile([P, d], f32, tag="mm")
                        for hd in range(nhd):
                            nc.tensor.matmul(
                                psp,
                                outT[:, 2 * hd:2 * hd + 2, j * P:(j + 1) * P],
                                wp8[:, 2 * hd:2 * hd + 2, :],
                                start=(hd == 0), stop=(hd == nhd - 1),
                                perf_mode=DR,
                            )
                        # x = gr * proj + (rgb + gate*bp)
                        x = ypool.tile([P, d], f32, tag="x")
                        nc.vector.scalar_tensor_tensor(
                            out=x, in0=psp, scalar=gr, in1=rgb_raw[j],
                            op0=mybir.AluOpType.mult, op1=mybir.AluOpType.add,
                        )
                        # LayerNorm
                        stats = small.tile([P, 6], f32, tag="stats")
                        nc.vector.bn_stats(out=stats, in_=x)
                        mv = small.tile([P, 2], f32, tag="mv")
                        nc.vector.bn_aggr(out=mv, in_=stats)
                        sd = small.tile([P, 1], f32, tag="sd")
                        nc.scalar.activation(
                            out=sd, in_=mv[:, 1:2], func=AF.Sqrt, bias=eps_sb)
                        rstd = small.tile([P, 1], f32, tag="rstd")
                        nc.vector.reciprocal(out=rstd, in_=sd)
                        nc.vector.tensor_scalar(
                            out=x, in0=x, scalar1=mv[:, 0:1], scalar2=rstd,
                            op0=mybir.AluOpType.subtract,
                            op1=mybir.AluOpType.mult,
                        )
                        nc.vector.tensor_mul(out=x, in0=x, in1=gamma_bc)
                        nc.vector.tensor_add(out=x, in0=x, in1=beta_bc)
                        nc.sync.dma_start(
                            out=out[b, q0 + j * P:q0 + (j + 1) * P, :], in_=x)

                if b + 1 < b_loc:
                    poseT = next_poseT

    nc.compile()
    return nc


_CACHE = {}


def kernel(**inputs):
    from concourse.bass_utils import run_bass_kernel_spmd

    if "nc" not in _CACHE:
        _CACHE["nc"] = build_nc()
    nc = _CACHE["nc"]

    weights = {k: np.ascontiguousarray(inputs[k], dtype=np.float32)
               for k in WEIGHT_NAMES}
    rgb = np.ascontiguousarray(inputs["rgb"], dtype=np.float32)
    pose = np.ascontiguousarray(inputs["pose"], dtype=np.float32)

    in_maps = []
    for i in range(N_CORES):
        m = dict(weights)
        m["rgb"] = np.ascontiguousarray(rgb[i * B_LOC:(i + 1) * B_LOC])
        m["pose"] = np.ascontiguousarray(pose[i * B_LOC:(i + 1) * B_LOC])
        in_maps.append(m)

    res = run_bass_kernel_spmd(nc, in_maps, list(range(N_CORES))).results
    return np.concatenate([res[i]["out"] for i in range(N_CORES)], axis=0)
